# revision 9
# baseline (speedup 1.0000x reference)
"""Trainium2 Bass kernel for nn_AttentionBlock (B=8, LN=2048, IDM=HDM=ODM=1024).

Sharding: data-parallel over batch, one batch element per NeuronCore (8 cores).

Math restructure (host precompute, fp64):
    W  = q @ k.T        so scores = (i@q) @ (i@k).T = i @ W @ i.T
    V2 = v @ mlp        so out    = lrelu(att @ (i@V2) + i@mlp) + bias
Per-core on-chip:
    A = i @ W           [ln, idm]   (transposed tiles: AT[e, s])
    scores = A @ i.T    [ln, ln]
    att = softmax(scores)
    U = i @ V2 ; M = i @ mlp
    out = lrelu(att @ U + M) + bias

Precision: the softmax amplifies score errors, so the QK path uses fp16
hi/lo pieces: main passes in fp16 (11-bit mantissa) plus the two cross
terms per matmul packed into fp8-e5m2 DoubleRow instructions (two K=128
contractions per instruction, ~2x rate). The value path (U/M/att@U) is
fp16 single-pass. All accumulation fp32 in PSUM.

Layout: contraction dim always on partitions. iT = i.T pieces come from
the host (ih fp16, i8 = [e5m2(i), e5m2(i - f16(i))] pair). A pieces are
derived on-chip from PSUM (Ah fp16 + A8 e5m2 pair). M is staged via DRAM
and re-added into the att@U PSUM group with an identity matmul.
"""
import numpy as np
import ml_dtypes

import concourse.bacc as bacc
import concourse.mybir as mybir
import concourse.tile as tile
from concourse import bass_utils

F32 = mybir.dt.float32
F16 = mybir.dt.float16
F8E5 = mybir.dt.float8e5
DR = mybir.MatmulPerfMode.DoubleRow
Act = mybir.ActivationFunctionType
Axis = mybir.AxisListType

LN = 2048      # sequence length
D = 1024       # idm = hdm = odm
N_CORES = 8
DC = D // 128      # 8 contraction chunks
ST = LN // 128     # 16 s-tiles
TB = LN // 512     # 4 t-blocks (N=512)
OB = D // 512      # 2 o-blocks
NEG_SLOPE = 0.2

_cached_nc = None


def _build():
    nc = bacc.Bacc("TRN2", target_bir_lowering=False, debug=False)

    ih = nc.dram_tensor("ih", [D, LN], F16, kind="ExternalInput")
    i8 = nc.dram_tensor("i8", [D, 2, LN], F8E5, kind="ExternalInput")
    wh = nc.dram_tensor("wh", [D, D], F16, kind="ExternalInput")
    w8 = nc.dram_tensor("w8", [D, 2, D], F8E5, kind="ExternalInput")
    v2h = nc.dram_tensor("v2h", [D, D], F16, kind="ExternalInput")
    mlph = nc.dram_tensor("mlph", [D, D], F16, kind="ExternalInput")
    bias = nc.dram_tensor("bias", [LN, D], F32, kind="ExternalInput")
    ident = nc.dram_tensor("ident", [128, 128], F16, kind="ExternalInput")
    out_d = nc.dram_tensor("out", [LN, D], F32, kind="ExternalOutput")

    # [D, X] viewed as [128 partitions, DC chunks, X]
    def pcv(t, x):
        return t.ap().rearrange("(c p) x -> p c x", p=128)

    def pcv2(t, x):  # [D, 2, X] -> [p, c, 2, X]
        return t.ap().rearrange("(c p) two x -> p c two x", p=128)

    ihv = pcv(ih, LN)
    i8v = pcv2(i8, LN)

    with tile.TileContext(nc) as tc:
        with tc.tile_pool(name="pers", bufs=1) as pers, \
             tc.tile_pool(name="dram", bufs=1, space="DRAM") as dram:
            ih_sb = pers.tile([128, DC, LN], F16)        # 32 KB/part
            i8_sb = pers.tile([128, DC, 2, LN], F8E5)    # 32 KB/part
            Ah_sb = pers.tile([128, DC, LN], F16)        # 32 KB/part
            A8_sb = pers.tile([128, DC, 2, LN], F8E5)    # 32 KB/part
            U_sb = pers.tile([128, ST, D], F16)          # 32 KB/part
            id_sb = pers.tile([128, 128], F16)
            alpha_ap = pers.tile([128, 1], F32)
            nc.vector.memset(alpha_ap, NEG_SLOPE)
            nc.sync.dma_start(out=id_sb, in_=ident.ap())

            M_d = dram.tile([ST, 128, D], F16)

            _psum_cm = tc.tile_pool(name="psum", bufs=1, space="PSUM")
            psum_pool = _psum_cm.__enter__()
            _ps_ctr = [0]
            _ps_tags = ["sc0", "sc1", "sc2", "sc3", "av0", "av1", "pa0", "pa1"]

            def prep_psum(name):
                tag = _ps_tags[_ps_ctr[0] % 8]
                _ps_ctr[0] += 1
                return psum_pool.tile([128, 512], F32, name=f"{name}{_ps_ctr[0]}",
                                      tag=tag)

            # ================= Phase A =================
            # Order: U (needs only ih+V2) runs first while w8/i8 stream in;
            # then A (wh streamed per-ec); then M (mlp reuses V2's slot).
            with tc.tile_pool(name="pa_w", bufs=1) as pa_w, \
                 tc.tile_pool(name="pa_whs", bufs=2) as pa_whs, \
                 tc.tile_pool(name="pa_m", bufs=2) as pa_m:
                v2_sb = pa_w.tile([128, DC, D], F16, name="v2_sb", tag="pav")
                w8_sb = pa_w.tile([128, DC, 2, D], F8E5, name="w8_sb", tag="pa8")
                v2v = pcv(v2h, D)
                for dc in range(DC):
                    nc.sync.dma_start(out=ih_sb[:, dc, 0:128], in_=ihv[:, dc, 0:128])
                for dc in range(DC):
                    nc.sync.dma_start(out=v2_sb[:, dc, 0:512], in_=v2v[:, dc, 0:512])
                for dc in range(DC):
                    nc.sync.dma_start(out=ih_sb[:, dc, 128:512], in_=ihv[:, dc, 128:512])
                for cb in range(1, 4):
                    c_sl = slice(cb * 512, cb * 512 + 512)
                    for dc in range(DC):
                        nc.sync.dma_start(out=ih_sb[:, dc, c_sl], in_=ihv[:, dc, c_sl])
                for dc in range(DC):
                    nc.sync.dma_start(out=v2_sb[:, dc, 512:1024], in_=v2v[:, dc, 512:1024])
                for dc in range(DC):
                    nc.sync.dma_start(out=w8_sb[:, dc], in_=pcv2(w8, D)[:, dc])
                    nc.sync.dma_start(out=i8_sb[:, dc], in_=i8v[:, dc])

                # --- U = i @ V2 -> [t, o] fp16 (stationary ih chunks)
                for ob in range(OB):
                    o_sl = slice(ob * 512, ob * 512 + 512)
                    for tc_ in range(ST):
                        t_sl = slice(tc_ * 128, tc_ * 128 + 128)
                        ps = prep_psum("pu")
                        for dc in range(DC):
                            nc.tensor.matmul(ps, ih_sb[:, dc, t_sl],
                                             v2_sb[:, dc, o_sl],
                                             start=(dc == 0), stop=(dc == DC - 1))
                        nc.vector.tensor_copy(U_sb[:, tc_, o_sl], ps)

                # --- A = i @ W  ->  AT[e, s] tiles, split to fp16 + e5m2 pair
                def a_group(wh_t, ec):
                    e_sl = slice(ec * 128, ec * 128 + 128)
                    for sb_ in range(TB):
                        ps = prep_psum("pa")
                        s_sl = slice(sb_ * 512, sb_ * 512 + 512)
                        for dc in range(DC):
                            nc.tensor.matmul(ps, wh_t[:, dc], ih_sb[:, dc, s_sl],
                                             start=(dc == 0), stop=False)
                        for dc in range(DC):
                            nc.tensor.matmul(ps, w8_sb[:, dc, :, e_sl],
                                             i8_sb[:, dc, :, s_sl],
                                             start=False, stop=(dc == DC - 1),
                                             perf_mode=DR)
                        nc.vector.tensor_copy(Ah_sb[:, ec, s_sl], ps)
                        nc.vector.tensor_sub(A8_sb[:, ec, 0, s_sl], ps,
                                             Ah_sb[:, ec, s_sl])
                        nc.vector.tensor_copy(A8_sb[:, ec, 1, s_sl], ps)

                pend = []
                for ec in range(DC):
                    wh_t = pa_whs.tile([128, DC, 128], F16, name="wh_t", tag="whs")
                    nc.sync.dma_start(out=wh_t, in_=pcv(wh, D)[:, :, ec * 128:ec * 128 + 128])
                    pend.append((wh_t, ec))
                    if len(pend) > 1:
                        a_group(*pend.pop(0))
                a_group(*pend.pop(0))

                # --- M = i @ mlp -> [s, o] fp16 -> DRAM staging
                mlp_sb = pa_w.tile([128, DC, D], F16, name="mlp_sb", tag="pav")
                for dc in range(DC):
                    nc.sync.dma_start(out=mlp_sb[:, dc], in_=pcv(mlph, D)[:, dc])
                for tc_ in range(ST):
                    t_sl = slice(tc_ * 128, tc_ * 128 + 128)
                    m_t = pa_m.tile([128, D], F16, name="m_t", tag="mst")
                    for ob in range(OB):
                        ps = prep_psum("pm")
                        o_sl = slice(ob * 512, ob * 512 + 512)
                        for dc in range(DC):
                            nc.tensor.matmul(ps, ih_sb[:, dc, t_sl],
                                             mlp_sb[:, dc, o_sl],
                                             start=(dc == 0), stop=(dc == DC - 1))
                        nc.vector.tensor_copy(m_t[:, o_sl], ps)
                    nc.sync.dma_start(out=M_d[tc_], in_=m_t)

            # ================= Phase B: scores/softmax/att@U =================
            with tc.tile_pool(name="pb_att", bufs=2) as pb_att, \
                 tc.tile_pool(name="pb_one", bufs=1) as pb_one, \
                 tc.tile_pool(name="pb_str", bufs=2) as pb_str, \
                 tc.tile_pool(name="pb_st", bufs=2) as pb_st:
                attT_t = pb_one.tile([128, ST, 512], F16, name="attT", tag="attT")

                def prefetch_out(si):
                    s_sl = slice(si * 128, si * 128 + 128)
                    m_t = pb_str.tile([128, D], F16, name="m_t", tag="mst")
                    nc.gpsimd.dma_start(out=m_t[:, 0:512], in_=M_d[si][:, 0:512])
                    nc.gpsimd.dma_start(out=m_t[:, 512:1024], in_=M_d[si][:, 512:1024])
                    bias_t = pb_str.tile([128, D], F32, name="bias_t", tag="bias")
                    nc.gpsimd.dma_start(out=bias_t[:, 0:512], in_=bias.ap()[s_sl, 0:512])
                    nc.gpsimd.dma_start(out=bias_t[:, 512:1024], in_=bias.ap()[s_sl, 512:1024])
                    return m_t, bias_t

                def scores_softmax(si):
                    st4 = si % 4
                    s_sl = slice(si * 128, si * 128 + 128)
                    scs = [
                        psum_pool.tile([128, 512], F32, name=f"sc{si}_{tb}",
                                       tag=f"sc{tb}")
                        for tb in range(TB)
                    ]
                    for ec in range(DC):
                        first = ec == 0
                        for tb in range(TB):
                            t_sl = slice(tb * 512, tb * 512 + 512)
                            nc.tensor.matmul(scs[tb], Ah_sb[:, ec, s_sl],
                                             ih_sb[:, ec, t_sl],
                                             start=first, stop=False)
                    for ec in range(DC):
                        last = ec == DC - 1
                        for tb in range(TB):
                            t_sl = slice(tb * 512, tb * 512 + 512)
                            nc.tensor.matmul(scs[tb], A8_sb[:, ec, :, s_sl],
                                             i8_sb[:, ec, :, t_sl],
                                             start=False, stop=last,
                                             perf_mode=DR)

                    # softmax: per-block max+exp, then algebraic rescale
                    st_t = pb_st.tile([128, 24], F32, name="st_t", tag="stats")
                    negm4 = st_t[:, 0:4]
                    sums = st_t[:, 4:8]
                    negM = st_t[:, 8:9]
                    S = st_t[:, 9:10]
                    recip = st_t[:, 10:11]
                    g4 = st_t[:, 12:16]
                    f4 = st_t[:, 16:20]
                    gs4 = st_t[:, 20:24]
                    att_t = pb_att.tile([128, LN], F16, name="att_t", tag="att")
                    for tb in range(TB):
                        t_sl = slice(tb * 512, tb * 512 + 512)
                        nc.vector.reduce_max(negm4[:, tb:tb + 1], scs[tb],
                                             axis=Axis.X, negate=True)
                        nc.scalar.activation(
                            out=att_t[:, t_sl], in_=scs[tb],
                            func=Act.Exp, bias=negm4[:, tb:tb + 1], scale=1.0,
                            accum_out=sums[:, tb:tb + 1],
                        )
                    nc.vector.tensor_reduce(negM, negm4, axis=Axis.X,
                                            op=mybir.AluOpType.min)
                    nc.scalar.activation(out=g4, in_=negm4, func=Act.Exp,
                                         bias=negM, scale=-1.0)
                    nc.vector.tensor_mul(gs4, g4, sums)
                    nc.vector.reduce_sum(S, gs4, axis=Axis.X)
                    nc.vector.reciprocal(recip, S)
                    nc.vector.tensor_scalar_mul(f4, g4, recip)

                    for tb in range(TB):
                        t_sl = slice(tb * 512, tb * 512 + 512)
                        nc.vector.tensor_scalar_mul(
                            att_t[:, t_sl], att_t[:, t_sl], f4[:, tb:tb + 1])
                    nc.sync.dma_start_transpose(
                        out=attT_t[:, :, st4 * 128:st4 * 128 + 128], in_=att_t)

                def av_out(si, m_t, bias_t):
                    st4 = si % 4
                    s_sl = slice(si * 128, si * 128 + 128)
                    out_t = pb_str.tile([128, D], F32, name="out_t", tag="out")
                    for ob in range(OB):
                        o_sl = slice(ob * 512, ob * 512 + 512)
                        ps = psum_pool.tile([128, 512], F32, name=f"av{si}_{ob}",
                                            tag=f"av{ob}")
                        for tc_ in range(ST):
                            nc.tensor.matmul(ps, attT_t[:, tc_, st4 * 128:st4 * 128 + 128],
                                             U_sb[:, tc_, o_sl],
                                             start=(tc_ == 0), stop=False)
                        nc.tensor.matmul(ps, id_sb, m_t[:, o_sl],
                                         start=False, stop=True)
                        nc.scalar.activation(
                            out=out_t[:, o_sl], in_=ps, func=Act.Prelu,
                            bias=0.0, scale=1.0, alpha=alpha_ap,
                        )
                        nc.vector.tensor_add(out_t[:, o_sl], out_t[:, o_sl],
                                             bias_t[:, o_sl])
                        nc.gpsimd.dma_start(out=out_d.ap()[s_sl, o_sl],
                                            in_=out_t[:, o_sl])

                pend_out = None
                for si in range(ST):
                    scores_softmax(si)
                    if pend_out is not None:
                        av_out(si - 1, *pend_out)
                    pend_out = prefetch_out(si)
                av_out(ST - 1, *pend_out)

            _psum_cm.__exit__(None, None, None)

    nc.compile()
    return nc


def _get_nc():
    global _cached_nc
    if _cached_nc is None:
        _cached_nc = _build()
    return _cached_nc


def _f16(x):
    return x.astype(np.float16)


def _e5(x):
    return x.astype(ml_dtypes.float8_e5m2)


def _prep_host(i, k, q, v, mlp, bias):
    W = (q.astype(np.float64) @ k.astype(np.float64).T).astype(np.float32)
    V2 = (v.astype(np.float64) @ mlp.astype(np.float64)).astype(np.float32)
    wh = _f16(W)
    wl32 = W - wh.astype(np.float32)
    w8 = np.stack([_e5(wl32), _e5(W)], axis=1)          # [D, 2, D]
    shared = dict(
        wh=wh, w8=w8, v2h=_f16(V2), mlph=_f16(mlp), bias=bias,
        ident=np.eye(128, dtype=np.float16),
    )
    in_maps = []
    for b in range(N_CORES):
        iT = np.ascontiguousarray(i[b].T)
        ih = _f16(iT)
        il32 = iT - ih.astype(np.float32)
        i8 = np.stack([_e5(iT), _e5(il32)], axis=1)     # [D, 2, LN]
        in_maps.append(dict(ih=ih, i8=i8, **shared))
    return in_maps


def kernel(i, k, q, v, mlp, bias):
    i = np.asarray(i, dtype=np.float32)
    k = np.asarray(k, dtype=np.float32)
    q = np.asarray(q, dtype=np.float32)
    v = np.asarray(v, dtype=np.float32)
    mlp = np.asarray(mlp, dtype=np.float32)
    bias = np.asarray(bias, dtype=np.float32)

    in_maps = _prep_host(i, k, q, v, mlp, bias)
    nc = _get_nc()
    res = bass_utils.run_bass_kernel_spmd(nc, in_maps, core_ids=list(range(N_CORES)))
    return np.stack([res.results[b]["out"] for b in range(N_CORES)])


# revision 10
# speedup vs baseline: 1.0168x; 1.0168x over previous
"""Trainium2 Bass kernel for nn_AttentionBlock (B=8, LN=2048, IDM=HDM=ODM=1024).

Sharding: data-parallel over batch, one batch element per NeuronCore (8 cores).

Math restructure (host precompute, fp64):
    W  = q @ k.T        so scores = (i@q) @ (i@k).T = i @ W @ i.T
    V2 = v @ mlp        so out    = lrelu(att @ (i@V2) + i@mlp) + bias
Per-core on-chip:
    A = i @ W           [ln, idm]   (transposed tiles: AT[e, s])
    scores = A @ i.T    [ln, ln]
    att = softmax(scores)
    U = i @ V2 ; M = i @ mlp
    out = lrelu(att @ U + M) + bias

Precision: the softmax amplifies score errors, so the QK path uses fp16
hi/lo pieces: main passes in fp16 (11-bit mantissa) plus the two cross
terms per matmul packed into fp8-e5m2 DoubleRow instructions (two K=128
contractions per instruction, ~2x rate). The value path (U/M/att@U) is
fp16 single-pass. All accumulation fp32 in PSUM.

Layout: contraction dim always on partitions. iT = i.T pieces come from
the host (ih fp16, i8 = [e5m2(i), e5m2(i - f16(i))] pair). A pieces are
derived on-chip from PSUM (Ah fp16 + A8 e5m2 pair). M is staged via DRAM
and re-added into the att@U PSUM group with an identity matmul.
"""
import numpy as np
import ml_dtypes

import concourse.bacc as bacc
import concourse.mybir as mybir
import concourse.tile as tile
from concourse import bass_utils

F32 = mybir.dt.float32
F16 = mybir.dt.float16
F8E5 = mybir.dt.float8e5
DR = mybir.MatmulPerfMode.DoubleRow
Act = mybir.ActivationFunctionType
Axis = mybir.AxisListType

LN = 2048      # sequence length
D = 1024       # idm = hdm = odm
N_CORES = 8
DC = D // 128      # 8 contraction chunks
ST = LN // 128     # 16 s-tiles
TB = LN // 512     # 4 t-blocks (N=512)
OB = D // 512      # 2 o-blocks
NEG_SLOPE = 0.2

_cached_nc = None


def _build():
    nc = bacc.Bacc("TRN2", target_bir_lowering=False, debug=False)

    ih = nc.dram_tensor("ih", [D, LN], F16, kind="ExternalInput")
    i8 = nc.dram_tensor("i8", [D, 2, LN], F8E5, kind="ExternalInput")
    wh = nc.dram_tensor("wh", [D, D], F16, kind="ExternalInput")
    w8 = nc.dram_tensor("w8", [D, 2, D], F8E5, kind="ExternalInput")
    v2h = nc.dram_tensor("v2h", [D, D], F16, kind="ExternalInput")
    mlph = nc.dram_tensor("mlph", [D, D], F16, kind="ExternalInput")
    bias = nc.dram_tensor("bias", [LN, D], F32, kind="ExternalInput")
    ident = nc.dram_tensor("ident", [128, 128], F16, kind="ExternalInput")
    out_d = nc.dram_tensor("out", [LN, D], F32, kind="ExternalOutput")

    # [D, X] viewed as [128 partitions, DC chunks, X]
    def pcv(t, x):
        return t.ap().rearrange("(c p) x -> p c x", p=128)

    def pcv2(t, x):  # [D, 2, X] -> [p, c, 2, X]
        return t.ap().rearrange("(c p) two x -> p c two x", p=128)

    ihv = pcv(ih, LN)
    i8v = pcv2(i8, LN)

    with tile.TileContext(nc) as tc:
        with tc.tile_pool(name="pers", bufs=1) as pers, \
             tc.tile_pool(name="dram", bufs=1, space="DRAM") as dram:
            ih_sb = pers.tile([128, DC, LN], F16)        # 32 KB/part
            i8_sb = pers.tile([128, DC, 2, LN], F8E5)    # 32 KB/part
            Ah_sb = pers.tile([128, DC, LN], F16)        # 32 KB/part
            A8_sb = pers.tile([128, DC, 2, LN], F8E5)    # 32 KB/part
            U_sb = pers.tile([128, ST, D], F16)          # 32 KB/part
            id_sb = pers.tile([128, 128], F16)
            alpha_ap = pers.tile([128, 1], F32)
            nc.vector.memset(alpha_ap, NEG_SLOPE)
            nc.sync.dma_start(out=id_sb, in_=ident.ap())

            M_d = dram.tile([ST, 128, D], F16)

            _psum_cm = tc.tile_pool(name="psum", bufs=1, space="PSUM")
            psum_pool = _psum_cm.__enter__()
            _ps_ctr = [0]
            _ps_tags = ["sc0", "sc1", "sc2", "sc3", "av0", "av1", "pa0", "pa1"]

            def prep_psum(name):
                tag = _ps_tags[_ps_ctr[0] % 8]
                _ps_ctr[0] += 1
                return psum_pool.tile([128, 512], F32, name=f"{name}{_ps_ctr[0]}",
                                      tag=tag)

            # ================= Phase A =================
            # Order: U (needs only ih+V2) runs first while w8/i8 stream in;
            # then A (wh streamed per-ec); then M (mlp reuses V2's slot).
            with tc.tile_pool(name="pa_w", bufs=1) as pa_w, \
                 tc.tile_pool(name="pa_whs", bufs=2) as pa_whs, \
                 tc.tile_pool(name="pa_m", bufs=2) as pa_m:
                v2_sb = pa_w.tile([128, DC, D], F16, name="v2_sb", tag="pav")
                w8_sb = pa_w.tile([128, DC, 2, D], F8E5, name="w8_sb", tag="pa8")
                v2v = pcv(v2h, D)
                for dc in range(DC):
                    nc.sync.dma_start(out=ih_sb[:, dc, 0:512], in_=ihv[:, dc, 0:512])
                    nc.sync.dma_start(out=v2_sb[:, dc, 0:512], in_=v2v[:, dc, 0:512])
                for cb in range(1, 4):
                    c_sl = slice(cb * 512, cb * 512 + 512)
                    for dc in range(DC):
                        nc.sync.dma_start(out=ih_sb[:, dc, c_sl], in_=ihv[:, dc, c_sl])
                for dc in range(DC):
                    nc.sync.dma_start(out=v2_sb[:, dc, 512:1024], in_=v2v[:, dc, 512:1024])
                for dc in range(DC):
                    nc.sync.dma_start(out=w8_sb[:, dc], in_=pcv2(w8, D)[:, dc])
                    nc.sync.dma_start(out=i8_sb[:, dc], in_=i8v[:, dc])

                # --- U = i @ V2 -> [t, o] fp16 (stationary ih chunks)
                for ob in range(OB):
                    o_sl = slice(ob * 512, ob * 512 + 512)
                    for tc_ in range(ST):
                        t_sl = slice(tc_ * 128, tc_ * 128 + 128)
                        ps = prep_psum("pu")
                        for dc in range(DC):
                            nc.tensor.matmul(ps, ih_sb[:, dc, t_sl],
                                             v2_sb[:, dc, o_sl],
                                             start=(dc == 0), stop=(dc == DC - 1))
                        nc.vector.tensor_copy(U_sb[:, tc_, o_sl], ps)

                # --- A = i @ W  ->  AT[e, s] tiles, split to fp16 + e5m2 pair
                def a_group(wh_t, ec):
                    e_sl = slice(ec * 128, ec * 128 + 128)
                    for sb_ in range(TB):
                        ps = prep_psum("pa")
                        s_sl = slice(sb_ * 512, sb_ * 512 + 512)
                        for dc in range(DC):
                            nc.tensor.matmul(ps, wh_t[:, dc], ih_sb[:, dc, s_sl],
                                             start=(dc == 0), stop=False)
                        for dc in range(DC):
                            nc.tensor.matmul(ps, w8_sb[:, dc, :, e_sl],
                                             i8_sb[:, dc, :, s_sl],
                                             start=False, stop=(dc == DC - 1),
                                             perf_mode=DR)
                        nc.vector.tensor_copy(Ah_sb[:, ec, s_sl], ps)
                        nc.vector.tensor_sub(A8_sb[:, ec, 0, s_sl], ps,
                                             Ah_sb[:, ec, s_sl])
                        nc.vector.tensor_copy(A8_sb[:, ec, 1, s_sl], ps)

                pend = []
                for ec in range(DC):
                    wh_t = pa_whs.tile([128, DC, 128], F16, name="wh_t", tag="whs")
                    nc.sync.dma_start(out=wh_t, in_=pcv(wh, D)[:, :, ec * 128:ec * 128 + 128])
                    pend.append((wh_t, ec))
                    if len(pend) > 1:
                        a_group(*pend.pop(0))
                a_group(*pend.pop(0))

                # --- M = i @ mlp -> [s, o] fp16 -> DRAM staging
                mlp_sb = pa_w.tile([128, DC, D], F16, name="mlp_sb", tag="pav")
                for dc in range(DC):
                    nc.sync.dma_start(out=mlp_sb[:, dc], in_=pcv(mlph, D)[:, dc])
                for tc_ in range(ST):
                    t_sl = slice(tc_ * 128, tc_ * 128 + 128)
                    m_t = pa_m.tile([128, D], F16, name="m_t", tag="mst")
                    for ob in range(OB):
                        ps = prep_psum("pm")
                        o_sl = slice(ob * 512, ob * 512 + 512)
                        for dc in range(DC):
                            nc.tensor.matmul(ps, ih_sb[:, dc, t_sl],
                                             mlp_sb[:, dc, o_sl],
                                             start=(dc == 0), stop=(dc == DC - 1))
                        nc.vector.tensor_copy(m_t[:, o_sl], ps)
                    nc.sync.dma_start(out=M_d[tc_], in_=m_t)

            # ================= Phase B: scores/softmax/att@U =================
            with tc.tile_pool(name="pb_att", bufs=2) as pb_att, \
                 tc.tile_pool(name="pb_one", bufs=1) as pb_one, \
                 tc.tile_pool(name="pb_str", bufs=2) as pb_str, \
                 tc.tile_pool(name="pb_st", bufs=2) as pb_st:
                attT_t = pb_one.tile([128, ST, 512], F16, name="attT", tag="attT")

                def prefetch_out(si):
                    s_sl = slice(si * 128, si * 128 + 128)
                    m_t = pb_str.tile([128, D], F16, name="m_t", tag="mst")
                    nc.gpsimd.dma_start(out=m_t[:, 0:512], in_=M_d[si][:, 0:512])
                    nc.gpsimd.dma_start(out=m_t[:, 512:1024], in_=M_d[si][:, 512:1024])
                    bias_t = pb_str.tile([128, D], F32, name="bias_t", tag="bias")
                    nc.gpsimd.dma_start(out=bias_t[:, 0:512], in_=bias.ap()[s_sl, 0:512])
                    nc.gpsimd.dma_start(out=bias_t[:, 512:1024], in_=bias.ap()[s_sl, 512:1024])
                    return m_t, bias_t

                def scores_softmax(si):
                    st4 = si % 4
                    s_sl = slice(si * 128, si * 128 + 128)
                    scs = [
                        psum_pool.tile([128, 512], F32, name=f"sc{si}_{tb}",
                                       tag=f"sc{tb}")
                        for tb in range(TB)
                    ]
                    for ec in range(DC):
                        first = ec == 0
                        for tb in range(TB):
                            t_sl = slice(tb * 512, tb * 512 + 512)
                            nc.tensor.matmul(scs[tb], Ah_sb[:, ec, s_sl],
                                             ih_sb[:, ec, t_sl],
                                             start=first, stop=False)
                    for ec in range(DC):
                        last = ec == DC - 1
                        for tb in range(TB):
                            t_sl = slice(tb * 512, tb * 512 + 512)
                            nc.tensor.matmul(scs[tb], A8_sb[:, ec, :, s_sl],
                                             i8_sb[:, ec, :, t_sl],
                                             start=False, stop=last,
                                             perf_mode=DR)

                    # softmax: per-block max+exp, then algebraic rescale
                    st_t = pb_st.tile([128, 24], F32, name="st_t", tag="stats")
                    negm4 = st_t[:, 0:4]
                    sums = st_t[:, 4:8]
                    negM = st_t[:, 8:9]
                    S = st_t[:, 9:10]
                    recip = st_t[:, 10:11]
                    g4 = st_t[:, 12:16]
                    f4 = st_t[:, 16:20]
                    gs4 = st_t[:, 20:24]
                    att_t = pb_att.tile([128, LN], F16, name="att_t", tag="att")
                    for tb in range(TB):
                        t_sl = slice(tb * 512, tb * 512 + 512)
                        nc.vector.reduce_max(negm4[:, tb:tb + 1], scs[tb],
                                             axis=Axis.X, negate=True)
                        nc.scalar.activation(
                            out=att_t[:, t_sl], in_=scs[tb],
                            func=Act.Exp, bias=negm4[:, tb:tb + 1], scale=1.0,
                            accum_out=sums[:, tb:tb + 1],
                        )
                    nc.vector.tensor_reduce(negM, negm4, axis=Axis.X,
                                            op=mybir.AluOpType.min)
                    nc.scalar.activation(out=g4, in_=negm4, func=Act.Exp,
                                         bias=negM, scale=-1.0)
                    nc.vector.tensor_mul(gs4, g4, sums)
                    nc.vector.reduce_sum(S, gs4, axis=Axis.X)
                    nc.vector.reciprocal(recip, S)
                    nc.vector.tensor_scalar_mul(f4, g4, recip)

                    for tb in range(TB):
                        t_sl = slice(tb * 512, tb * 512 + 512)
                        nc.vector.tensor_scalar_mul(
                            att_t[:, t_sl], att_t[:, t_sl], f4[:, tb:tb + 1])
                    nc.sync.dma_start_transpose(
                        out=attT_t[:, :, st4 * 128:st4 * 128 + 128], in_=att_t)

                def av_out(si, m_t, bias_t):
                    st4 = si % 4
                    s_sl = slice(si * 128, si * 128 + 128)
                    out_t = pb_str.tile([128, D], F32, name="out_t", tag="out")
                    for ob in range(OB):
                        o_sl = slice(ob * 512, ob * 512 + 512)
                        ps = psum_pool.tile([128, 512], F32, name=f"av{si}_{ob}",
                                            tag=f"av{ob}")
                        for tc_ in range(ST):
                            nc.tensor.matmul(ps, attT_t[:, tc_, st4 * 128:st4 * 128 + 128],
                                             U_sb[:, tc_, o_sl],
                                             start=(tc_ == 0), stop=False)
                        nc.tensor.matmul(ps, id_sb, m_t[:, o_sl],
                                         start=False, stop=True)
                        nc.scalar.activation(
                            out=out_t[:, o_sl], in_=ps, func=Act.Prelu,
                            bias=0.0, scale=1.0, alpha=alpha_ap,
                        )
                        nc.vector.tensor_add(out_t[:, o_sl], out_t[:, o_sl],
                                             bias_t[:, o_sl])
                        nc.gpsimd.dma_start(out=out_d.ap()[s_sl, o_sl],
                                            in_=out_t[:, o_sl])

                pend_out = None
                for si in range(ST):
                    scores_softmax(si)
                    if pend_out is not None:
                        av_out(si - 1, *pend_out)
                    pend_out = prefetch_out(si)
                av_out(ST - 1, *pend_out)

            _psum_cm.__exit__(None, None, None)

    nc.compile()
    return nc


def _get_nc():
    global _cached_nc
    if _cached_nc is None:
        _cached_nc = _build()
    return _cached_nc


def _f16(x):
    return x.astype(np.float16)


def _e5(x):
    return x.astype(ml_dtypes.float8_e5m2)


def _prep_host(i, k, q, v, mlp, bias):
    W = (q.astype(np.float64) @ k.astype(np.float64).T).astype(np.float32)
    V2 = (v.astype(np.float64) @ mlp.astype(np.float64)).astype(np.float32)
    wh = _f16(W)
    wl32 = W - wh.astype(np.float32)
    w8 = np.stack([_e5(wl32), _e5(W)], axis=1)          # [D, 2, D]
    shared = dict(
        wh=wh, w8=w8, v2h=_f16(V2), mlph=_f16(mlp), bias=bias,
        ident=np.eye(128, dtype=np.float16),
    )
    in_maps = []
    for b in range(N_CORES):
        iT = np.ascontiguousarray(i[b].T)
        ih = _f16(iT)
        il32 = iT - ih.astype(np.float32)
        i8 = np.stack([_e5(iT), _e5(il32)], axis=1)     # [D, 2, LN]
        in_maps.append(dict(ih=ih, i8=i8, **shared))
    return in_maps


def kernel(i, k, q, v, mlp, bias):
    i = np.asarray(i, dtype=np.float32)
    k = np.asarray(k, dtype=np.float32)
    q = np.asarray(q, dtype=np.float32)
    v = np.asarray(v, dtype=np.float32)
    mlp = np.asarray(mlp, dtype=np.float32)
    bias = np.asarray(bias, dtype=np.float32)

    in_maps = _prep_host(i, k, q, v, mlp, bias)
    nc = _get_nc()
    res = bass_utils.run_bass_kernel_spmd(nc, in_maps, core_ids=list(range(N_CORES)))
    return np.stack([res.results[b]["out"] for b in range(N_CORES)])
